# revision 16
# baseline (speedup 1.0000x reference)
"""Non-local block (embedded-gaussian, maxpool-subsampled keys/values) on 8 TRN2 cores.

Sharding: core c handles batch b=c//2, query-slab s=c%2 (T-slices 0-3 / 4-7,
4096 query positions each). phi/g convs are computed at full resolution per
batch on both of the batch's cores (maxpool happens after the conv, so both
cores need the full image); theta/scores/softmax/PV/final-conv are sharded.

Math per core (all matmuls bf16 with fp32 PSUM accumulation):
  theta = Wt x_slab + bt                    [Ci=128, 4096]
  phi   = pool(Wp x_full + bp)              [Ci, 2048]
  g     = pool(Wg x_full + bg)              [Ci, 2048] -> gT [2048, Ci]
  S^T   = phi^T theta (per 128-key chunk)   [keys, q] tiles
  P^T   = exp(S^T)  (no max subtraction: |S| <~ 70, exp fits fp32)
  yT    = sum_kt gT_kt^T @ P^T_kt           [Ci, q]  (PSUM accumulation)
  sums  = ones^T @ P^T_kt (PSUM accum)      [1, q]   (softmax denominators)
  out   = (Wf*inv)^T yT/sums + (x_slab + bias_total)   (BN scale folded into
          Wf, BN bias + conv bias + residual folded into xres host-side)
"""
import sys
sys.path.insert(0, '/opt/trn_rl_repo')
import numpy as np
import ml_dtypes
import concourse.bass as bass
import concourse.mybir as mybir
from concourse.tile import TileContext

FP32 = mybir.dt.float32
BF16 = mybir.dt.bfloat16
FP16 = mybir.dt.float16
AF = mybir.ActivationFunctionType
ALU = mybir.AluOpType
AX = mybir.AxisListType
BF16NP = ml_dtypes.bfloat16
FP16NP = np.float16

B, C, T, H, W = 4, 256, 8, 32, 32
CI = 128
N = T * H * W          # 8192
NL = N // 2            # 4096 per-core query slab
N2 = T * (H // 2) * (W // 2)  # 2048 pooled keys
KT = N2 // 128         # 16 key chunks
QB = 1024              # query block (PSUM-sized)
NQB = NL // QB         # 4
QC = 512               # matmul free-dim chunk


def split_excess_waits(nc):
    """This container's walrus caps sem waits per instruction; Tile's tail/loop
    drains exceed it. Spill excess waits onto preceding NoOp carriers."""
    n_split = 0
    for f in nc.m.functions:
        for bb in f.blocks:
            new = []
            changed = False
            for ins in bb.instructions:
                si = ins.sync_info
                cap = 2 if isinstance(ins, mybir.InstEventSemaphore) else 1
                if si is not None and si.on_wait and len(si.on_wait) > cap:
                    waits = list(si.on_wait)
                    for i, w in enumerate(waits[cap:]):
                        new.append(mybir.InstNoOp(
                            name=f"{ins.name}-wsplit{i}",
                            engine=ins.engine,
                            sync_info=mybir.SyncInfo(on_wait=[w], on_update=[]),
                            bass_nofuse=True,
                        ))
                    ins.sync_info = mybir.SyncInfo(
                        on_wait=waits[:cap], on_update=list(si.on_update))
                    changed = True
                    n_split += 1
                new.append(ins)
            if changed:
                bb.instructions = new
    return n_split


def build_nc(loop_n=0):
    """Build the per-core Tile program. loop_n>0 wraps the body in a hardware
    loop (for device-side timing via loop-count differencing)."""
    nc = bass.Bass()
    dp = nc.declare_dram_parameter
    xb_d = dp("xb", [2, 128, N], FP16, isOutput=False)        # full image, C-chunks
    xsb_d = dp("xsb", [2, 128, NL], FP16, isOutput=False)     # query slab (fp16)
    xres_d = dp("xres", [2, 128, NL], FP32, isOutput=False)   # slab + total bias (f32)
    wth_d = dp("wthT", [2, 128, 128], FP16, isOutput=False)
    wph_d = dp("wphT", [2, 128, 128], FP16, isOutput=False)
    wg_d = dp("wgT", [2, 128, 128], FP16, isOutput=False)
    wfin_d = dp("wfinT", [128, 256], FP16, isOutput=False)    # w_final.T * inv[o]
    bth_d = dp("b_th", [128, 1], FP32, isOutput=False)
    bph_d = dp("b_ph", [128, 1], FP32, isOutput=False)
    bg_d = dp("b_g", [128, 1], FP32, isOutput=False)
    ones_d = dp("ones_bf", [128, 128], BF16, isOutput=False)
    onesf_d = dp("ones_f", [1, 128], FP32, isOutput=False)
    ident_d = dp("ident", [128, 128], BF16, isOutput=False)
    out_d = dp("out", [2, 128, NL], FP32, isOutput=True)



    with TileContext(nc) as tc:
        from contextlib import ExitStack
        with tc.tile_pool(name="const", bufs=1) as cp:
            xb = [cp.tile([128, N], FP16, name=f"xb{i}") for i in range(2)]
            xsb = [cp.tile([128, NL], FP16, name=f"xsb{i}") for i in range(2)]
            xres = [cp.tile([128, NL], FP32, name=f"xres{i}") for i in range(2)]
            wth = [cp.tile([128, 128], FP16, name=f"wth{i}") for i in range(2)]
            wph = [cp.tile([128, 128], FP16, name=f"wph{i}") for i in range(2)]
            wg = [cp.tile([128, 128], FP16, name=f"wg{i}") for i in range(2)]
            wfin = cp.tile([128, 256], FP16, name="wfin")
            bth = cp.tile([128, 1], FP32, name="bth")
            bph = cp.tile([128, 1], FP32, name="bph")
            bg = cp.tile([128, 1], FP32, name="bg")
            ones = cp.tile([128, 128], BF16, name="ones")
            onesf = cp.tile([1, 128], FP32, name="onesf")
            ident = cp.tile([128, 128], BF16, name="ident")
            theta = cp.tile([128, NL], FP16, name="theta")
            phi_p = cp.tile([128, N2], FP16, name="phi_p")
            g_p = cp.tile([128, N2], BF16, name="g_p")
            gT = cp.tile([128, N2], BF16, name="gT")
            yraw = cp.tile([128, NL], FP32, name="yraw")
            yT = cp.tile([128, NL], FP16, name="yT")
            sums_sb = cp.tile([32, NL], FP32, name="sums_sb")
            rrow = cp.tile([32, NL], FP32, name="rrow")

            def body():
                # ---- input DMAs ----
                for i in range(2):
                    nc.sync.dma_start(xb[i][:], xb_d[i])
                    nc.sync.dma_start(xsb[i][:], xsb_d[i])
                    nc.sync.dma_start(xres[i][:], xres_d[i])
                    nc.sync.dma_start(wth[i][:], wth_d[i])
                    nc.sync.dma_start(wph[i][:], wph_d[i])
                    nc.sync.dma_start(wg[i][:], wg_d[i])
                nc.sync.dma_start(wfin[:], wfin_d[:])
                nc.sync.dma_start(bth[:], bth_d[:])
                nc.sync.dma_start(bph[:], bph_d[:])
                nc.sync.dma_start(bg[:], bg_d[:])
                nc.sync.dma_start(ones[:], ones_d[:])
                nc.sync.dma_start(onesf[:], onesf_d[:])
                nc.sync.dma_start(ident[:], ident_d[:])

                # ---- stage A: convs + pooling + transposes ----
                with tc.tile_pool(name="psA", bufs=3, space="PSUM") as psA, \
                     tc.tile_pool(name="wpool", bufs=3) as wpp:
                    # theta conv: [Ci, NL]
                    for nq in range(NL // QC):
                        ps = psA.tile([128, QC], FP32, tag="conv",
                                      padded_shape=[128, QB], name="psth")
                        for cc in range(2):
                            nc.tensor.matmul(
                                ps[:], wth[cc][:],
                                xsb[cc][:, nq * QC:(nq + 1) * QC],
                                start=(cc == 0), stop=(cc == 1))
                        nc.scalar.activation(
                            theta[:, nq * QC:(nq + 1) * QC], ps[:],
                            AF.Identity, bias=bth[:, 0:1])

                    # phi/g convs at full res + 2x2 maxpool
                    for (wt, bias_t, dst, pdt) in ((wph, bph, phi_p, FP16),
                                                   (wg, bg, g_p, BF16)):
                        for t in range(T):
                            ps = psA.tile([128, 1024], FP32, tag="conv",
                                          name="psconv")
                            for half in range(2):
                                sl = slice(t * 1024 + half * QC,
                                           t * 1024 + (half + 1) * QC)
                                for cc in range(2):
                                    nc.tensor.matmul(
                                        ps[:, half * QC:(half + 1) * QC],
                                        wt[cc][:], xb[cc][:, sl],
                                        start=(cc == 0), stop=(cc == 1))
                            # W-pool: max over adjacent w pairs (DVE, psum src)
                            wpt = wpp.tile([128, 512], pdt, tag="wp", name="wpt")
                            nc.vector.tensor_reduce(
                                out=wpt[:],
                                in_=ps[:].rearrange("p (a two) -> p a two", two=2),
                                axis=AX.X, op=ALU.max)
                            # H-pool: max over adjacent h pairs
                            wv = wpt[:].rearrange("p (h w) -> p h w", w=16)
                            nc.vector.tensor_max(
                                dst[:, t * 256:(t + 1) * 256]
                                    .rearrange("p (h w) -> p h w", w=16),
                                wv[:, 0:32:2, :], wv[:, 1:32:2, :])
                        # channel bias (commutes with spatial max)
                        nc.vector.tensor_scalar_add(dst[:], dst[:], bias_t[:, 0:1])

                    # gT: transpose pooled g, 128x128 chunks
                    for kt in range(KT):
                        pst = psA.tile([128, 128], BF16, tag="tr", bufs=2,
                                       name="pstr")
                        nc.tensor.transpose(
                            pst[:], g_p[:, kt * 128:(kt + 1) * 128], ident[:])
                        nc.vector.tensor_copy(
                            gT[:, kt * 128:(kt + 1) * 128], pst[:])

                # ---- stage B: attention (S^T -> exp -> PV + sums) ----
                with tc.tile_pool(name="psS", bufs=2, space="PSUM") as psS, \
                     tc.tile_pool(name="psY", bufs=1, space="PSUM") as psY, \
                     tc.tile_pool(name="psSum", bufs=2, space="PSUM") as psM, \
                     tc.tile_pool(name="ptp", bufs=3) as ptp:
                    nc.vector.memset(sums_sb[:], 1.0)
                    for qb in range(NQB):
                        q0 = qb * QB
                        ya = psY.tile([128, QB], FP32, tag="y", name="ya")
                        s0 = psM.tile([1, QC], FP32, tag="sum", name="s0")
                        s1 = psM.tile([1, QC], FP32, tag="sum", name="s1")
                        for kt in range(KT):
                            k0 = kt * 128
                            ss = psS.tile([128, QB], FP32, tag="S", name="ss")
                            for h in range(2):
                                nc.tensor.matmul(
                                    ss[:, h * QC:(h + 1) * QC],
                                    phi_p[:, k0:k0 + 128],
                                    theta[:, q0 + h * QC:q0 + (h + 1) * QC],
                                    start=True, stop=True)
                            ptt = ptp.tile([128, QB], BF16, tag="pt", name="ptt")
                            nc.scalar.activation(ptt[:], ss[:], AF.Exp)
                            for h in range(2):
                                nc.tensor.matmul(
                                    ya[:, h * QC:(h + 1) * QC],
                                    gT[:, k0:k0 + 128],
                                    ptt[:, h * QC:(h + 1) * QC],
                                    start=(kt == 0), stop=(kt == KT - 1))
                            nc.tensor.matmul(
                                s0[:], ones[:, 0:1], ptt[:, 0:QC],
                                start=(kt == 0), stop=(kt == KT - 1))
                            nc.tensor.matmul(
                                s1[:], ones[:, 0:1], ptt[:, QC:QB],
                                start=(kt == 0), stop=(kt == KT - 1))
                        # stash sums, evac y
                        for j, sm in enumerate((s0, s1)):
                            off = q0 + j * QC
                            nc.vector.tensor_copy(
                                sums_sb[0:1, off:off + QC], sm[:])
                        nc.scalar.copy(yraw[:, q0:q0 + QB], ya[:])

                # ---- softmax denominators: exact 1/sums via DVE 32x32 block
                # transpose (spread the single-partition row across 32 lanes so
                # the 8-cycle/elem divide runs 32-wide instead of 1-wide) ----
                with tc.tile_pool(name="rcp", bufs=2) as rcp:
                    for ch in range(NQB):
                        c0 = ch * QB
                        sp1 = rcp.tile([32, QB], FP32, tag="sp", name="sp1")
                        sp2 = rcp.tile([32, QB], FP32, tag="sp", name="sp2")
                        nc.vector.transpose(sp1[:], sums_sb[0:32, c0:c0 + QB])
                        nc.vector.memset(sp2[:], 1.0)
                        nc.vector.reciprocal(sp2[:, 0:QB:32], sp1[:, 0:QB:32])
                        nc.vector.transpose(rrow[0:32, c0:c0 + QB], sp2[:])

                # ---- normalize y, final conv, residual ----
                with tc.tile_pool(name="psR", bufs=2, space="PSUM") as psR, \
                     tc.tile_pool(name="psF", bufs=3, space="PSUM") as psF, \
                     tc.tile_pool(name="outp", bufs=3) as outp:
                    for qc in range(NL // QC):
                        rps = psR.tile([128, QC], FP32, tag="R", name="rps")
                        nc.tensor.matmul(
                            rps[:], onesf[0:1, :],
                            rrow[0:1, qc * QC:(qc + 1) * QC],
                            start=True, stop=True)
                        nc.vector.tensor_mul(
                            yT[:, qc * QC:(qc + 1) * QC],
                            yraw[:, qc * QC:(qc + 1) * QC], rps[:])
                    for oc in range(2):
                        for qc in range(NL // QC):
                            pf = psF.tile([128, QC], FP32, tag="F", name="pf")
                            nc.tensor.matmul(
                                pf[:], wfin[:, oc * 128:(oc + 1) * 128],
                                yT[:, qc * QC:(qc + 1) * QC],
                                start=True, stop=True)
                            ot = outp.tile([128, QC], FP32, tag="ot", name="ot")
                            nc.vector.tensor_add(
                                ot[:], xres[oc][:, qc * QC:(qc + 1) * QC], pf[:])
                            nc.sync.dma_start(
                                out_d[oc, :, qc * QC:(qc + 1) * QC], ot[:])

            if loop_n:
                with tc.For_i(0, loop_n, 1):
                    body()
            else:
                body()

    split_excess_waits(nc)
    return nc


def prep_inputs(x, w_theta, b_theta, w_phi, b_phi, w_g, b_g,
                w_final, b_final, bn_gamma, bn_beta, bn_mean, bn_var):
    """Host-side prep -> list of 8 per-core input dicts."""
    BN_EPS = 1e-5
    x = np.asarray(x, np.float32)
    inv = (np.asarray(bn_gamma, np.float32)
           / np.sqrt(np.asarray(bn_var, np.float32) + BN_EPS))
    # total additive bias on the output channel o:
    # BN(conv + b_final) = inv*conv + (inv*(b_final - mean) + beta)
    bias_tot = inv * (np.asarray(b_final, np.float32)
                      - np.asarray(bn_mean, np.float32)) + np.asarray(bn_beta, np.float32)
    wfinT = (np.asarray(w_final, np.float32) * inv[:, None]).T  # [Ci, C]

    def cchunk(a2d):  # [C, K] -> [2, 128, K]
        return np.ascontiguousarray(a2d.reshape(2, 128, -1))

    wthT = np.ascontiguousarray(np.asarray(w_theta, np.float32).T)  # [C, Ci]
    wphT = np.ascontiguousarray(np.asarray(w_phi, np.float32).T)
    wgT = np.ascontiguousarray(np.asarray(w_g, np.float32).T)

    common = {
        "wthT": cchunk(wthT).astype(FP16NP),
        "wphT": cchunk(wphT).astype(FP16NP),
        "wgT": cchunk(wgT).astype(FP16NP),
        "wfinT": np.ascontiguousarray(wfinT).astype(FP16NP),
        "b_th": np.asarray(b_theta, np.float32).reshape(128, 1),
        "b_ph": np.asarray(b_phi, np.float32).reshape(128, 1),
        "b_g": np.asarray(b_g, np.float32).reshape(128, 1),
        "ones_bf": np.ones((128, 128), BF16NP),
        "ones_f": np.ones((1, 128), np.float32),
        "ident": np.eye(128, dtype=BF16NP),
    }
    in_maps = []
    for c in range(8):
        b, s = c // 2, c % 2
        xf = x[b].reshape(C, N)
        xs = xf[:, s * NL:(s + 1) * NL]
        m = dict(common)
        m["xb"] = cchunk(xf).astype(FP16NP)
        m["xsb"] = cchunk(xs).astype(FP16NP)
        m["xres"] = np.ascontiguousarray(
            (xs + bias_tot[:, None]).reshape(2, 128, NL)).astype(np.float32)
        in_maps.append(m)
    return in_maps


def assemble(results):
    """results: list of 8 dicts with 'out' [2,128,NL] f32 -> [B,C,T,H,W]."""
    out = np.empty((B, C, N), np.float32)
    for c in range(8):
        b, s = c // 2, c % 2
        out[b, :, s * NL:(s + 1) * NL] = results[c]["out"].reshape(C, NL)
    return out.reshape(B, C, T, H, W)


_NC_CACHE = {}


def kernel(**inputs):
    from concourse.bass_utils import run_bass_kernel_spmd
    if "main" not in _NC_CACHE:
        _NC_CACHE["main"] = build_nc()
    nc = _NC_CACHE["main"]
    in_maps = prep_inputs(**inputs)
    res = run_bass_kernel_spmd(nc, in_maps, list(range(8)))
    return assemble(res.results)


# revision 31
# speedup vs baseline: 1.5236x; 1.5236x over previous
"""Non-local block (embedded-gaussian, maxpool-subsampled keys/values) on 8 TRN2 cores.

Sharding: core c handles batch b=c//2, query-slab s=c%2 (4096 query positions).
The per-core image xb is ROTATED so the core's own slab occupies columns
0:4096 (t-slice rotation only permutes pooled keys, and attention is
key-permutation invariant), letting one SPMD program serve all cores.
phi/g convs run at full resolution per batch on both of the batch's cores
(maxpool follows the conv, so both cores need the full image).

Per-core math (matmuls fp16/bf16 with fp32 PSUM accumulation):
  theta = Wt x[:, :4096] + bt            [Ci=128, 4096]        (fp16)
  phi   = pool(Wp x + bp)                [Ci, 2048]            (fp16)
  g     = pool(Wg x + bg)                [Ci, 2048] -> gT [2048, Ci] (bf16)
  S^T   = phi^T theta per 128-key chunk  (fp16 x fp16 -> fp32 PSUM)
  P^T   = exp(S^T)  (bf16: values reach e^78, beyond fp16 range;
          no max subtraction needed: |S| < 80 so exp fits fp32)
  yT    = sum_kt gT_kt^T @ P^T_kt        (PSUM accumulation)
  sums  = ones^T @ P^T_kt (PSUM accum)   [1, q] softmax denominators
  out   = (Wf*inv)^T (yT/sums) + (x_slab + total_bias)
          (BN scale folded into Wf; BN bias + final conv bias + residual
           folded into xres host-side)
The whole normalize/final-conv/output tail runs per query-block, overlapped
with the next block's attention.
"""
import sys
sys.path.insert(0, '/opt/trn_rl_repo')
import numpy as np
import ml_dtypes
import concourse.bass as bass
import concourse.mybir as mybir
from concourse.tile import TileContext

FP32 = mybir.dt.float32
BF16 = mybir.dt.bfloat16
FP16 = mybir.dt.float16
AF = mybir.ActivationFunctionType
ALU = mybir.AluOpType
AX = mybir.AxisListType
BF16NP = ml_dtypes.bfloat16
FP16NP = np.float16

B, C, T, H, W = 4, 256, 8, 32, 32
CI = 128
N = T * H * W          # 8192
NL = N // 2            # 4096 per-core query slab
N2 = T * (H // 2) * (W // 2)  # 2048 pooled keys
KT = N2 // 128         # 16 key chunks
QB = 1024              # query block (PSUM-sized)
NQB = NL // QB         # 4
QC = 512               # matmul free-dim chunk


def split_excess_waits(nc):
    """This container's walrus caps sem waits per instruction; Tile's tail/loop
    drains exceed it. Spill excess waits onto preceding NoOp carriers."""
    n_split = 0
    for f in nc.m.functions:
        for bb in f.blocks:
            new = []
            changed = False
            for ins in bb.instructions:
                si = ins.sync_info
                cap = 2 if isinstance(ins, mybir.InstEventSemaphore) else 1
                if si is not None and si.on_wait and len(si.on_wait) > cap:
                    waits = list(si.on_wait)
                    for i, w in enumerate(waits[cap:]):
                        new.append(mybir.InstNoOp(
                            name=f"{ins.name}-wsplit{i}",
                            engine=ins.engine,
                            sync_info=mybir.SyncInfo(on_wait=[w], on_update=[]),
                            bass_nofuse=True,
                        ))
                    ins.sync_info = mybir.SyncInfo(
                        on_wait=waits[:cap], on_update=list(si.on_update))
                    changed = True
                    n_split += 1
                new.append(ins)
            if changed:
                bb.instructions = new
    return n_split


def build_nc(loop_n=0):
    """Build the per-core Tile program. loop_n>0 wraps the body in a hardware
    loop (device-side timing via loop-count differencing)."""
    nc = bass.Bass()
    dp = nc.declare_dram_parameter
    xb_d = dp("xb", [2, 128, N], FP16, isOutput=False)        # rotated image
    xres_d = dp("xres", [2, 128, NL], FP32, isOutput=False)   # slab + total bias
    wth_d = dp("wthT", [2, 128, 128], FP16, isOutput=False)
    wph_d = dp("wphT", [2, 128, 128], FP16, isOutput=False)
    wg_d = dp("wgT", [2, 128, 128], FP16, isOutput=False)
    wfin_d = dp("wfinT", [128, 256], FP16, isOutput=False)    # w_final.T * inv[o]
    bth_d = dp("b_th", [128, 1], FP32, isOutput=False)
    bph_d = dp("b_ph", [128, 1], FP32, isOutput=False)
    bg_d = dp("b_g", [128, 1], FP32, isOutput=False)
    ones_d = dp("ones_bf", [128, 128], BF16, isOutput=False)
    onesf_d = dp("ones_f", [1, 128], FP32, isOutput=False)
    ident_d = dp("ident", [128, 128], BF16, isOutput=False)
    out_d = dp("out", [2, 128, NL], FP32, isOutput=True)

    with TileContext(nc) as tc:
        with tc.tile_pool(name="const", bufs=1) as cp:
            xb = [cp.tile([128, N], FP16, name=f"xb{i}") for i in range(2)]
            xres = [cp.tile([128, NL], FP32, name=f"xres{i}") for i in range(2)]
            wth = [cp.tile([128, 128], FP16, name=f"wth{i}") for i in range(2)]
            wph = [cp.tile([128, 128], FP16, name=f"wph{i}") for i in range(2)]
            wg = [cp.tile([128, 128], FP16, name=f"wg{i}") for i in range(2)]
            wfin = cp.tile([128, 256], FP16, name="wfin")
            bth = cp.tile([128, 1], FP32, name="bth")
            bph = cp.tile([128, 1], FP32, name="bph")
            bg = cp.tile([128, 1], FP32, name="bg")
            ones = cp.tile([128, 128], BF16, name="ones")
            onesf = cp.tile([1, 128], FP32, name="onesf")
            ident = cp.tile([128, 128], BF16, name="ident")
            theta = cp.tile([128, NL], FP16, name="theta")
            phi_p = cp.tile([128, N2], FP16, name="phi_p")
            g_p = cp.tile([128, N2], BF16, name="g_p")
            gT = cp.tile([128, N2], BF16, name="gT")
            yraw = cp.tile([128, NL], FP32, name="yraw")
            yT = cp.tile([128, NL], FP16, name="yT")
            sums_sb = cp.tile([32, NL], FP32, name="sums_sb")
            rrow = cp.tile([32, NL], FP32, name="rrow")

            def body():
                # ---- input DMAs (small/load-bearing first) ----
                for i in range(2):
                    nc.sync.dma_start(wth[i][:], wth_d[i])
                    nc.sync.dma_start(wph[i][:], wph_d[i])
                    nc.sync.dma_start(wg[i][:], wg_d[i])
                nc.sync.dma_start(bth[:], bth_d[:])
                nc.sync.dma_start(bph[:], bph_d[:])
                nc.sync.dma_start(bg[:], bg_d[:])
                nc.sync.dma_start(ones[:], ones_d[:])
                nc.sync.dma_start(onesf[:], onesf_d[:])
                nc.sync.dma_start(ident[:], ident_d[:])
                nc.sync.dma_start(wfin[:], wfin_d[:])
                # image in quarters: theta conv needs only the first (slab)
                NQ4 = N // 4
                for h in range(4):
                    for i in range(2):
                        nc.sync.dma_start(
                            xb[i][:, h * NQ4:(h + 1) * NQ4],
                            xb_d[i, :, h * NQ4:(h + 1) * NQ4])
                for i in range(2):
                    nc.sync.dma_start(xres[i][:], xres_d[i])

                # ---- stage A: convs + pooling + transposes ----
                with tc.tile_pool(name="psA", bufs=3, space="PSUM") as psA, \
                     tc.tile_pool(name="wpool", bufs=3) as wpp:
                    # theta conv first (rotated layout: slab = cols 0:NL)
                    for nq in range(NL // QC):
                        ps = psA.tile([128, QC], FP32, tag="conv",
                                      padded_shape=[128, 1024], name="psth")
                        for cc in range(2):
                            nc.tensor.matmul(
                                ps[:], wth[cc][:],
                                xb[cc][:, nq * QC:(nq + 1) * QC],
                                start=(cc == 0), stop=(cc == 1))
                        # evac + bias on DVE (ACT is the exp-bound engine)
                        nc.vector.tensor_scalar_add(
                            theta[:, nq * QC:(nq + 1) * QC], ps[:],
                            bth[:, 0:1])

                    # phi/g convs at full res + 2x2 maxpool
                    for (wt, bias_t, dst, pdt) in ((wph, bph, phi_p, FP16),
                                                   (wg, bg, g_p, BF16)):
                        for t in range(T):
                            ps = psA.tile([128, 1024], FP32, tag="conv",
                                          name="psconv")
                            for half in range(2):
                                sl = slice(t * 1024 + half * QC,
                                           t * 1024 + (half + 1) * QC)
                                for cc in range(2):
                                    nc.tensor.matmul(
                                        ps[:, half * QC:(half + 1) * QC],
                                        wt[cc][:], xb[cc][:, sl],
                                        start=(cc == 0), stop=(cc == 1))
                            # W-pool: max over adjacent w pairs (DVE, psum src)
                            wpt = wpp.tile([128, 512], pdt, tag="wp", name="wpt")
                            nc.vector.tensor_reduce(
                                out=wpt[:],
                                in_=ps[:].rearrange("p (a two) -> p a two", two=2),
                                axis=AX.X, op=ALU.max)
                            # H-pool: max over adjacent h pairs
                            wv = wpt[:].rearrange("p (h w) -> p h w", w=16)
                            nc.vector.tensor_max(
                                dst[:, t * 256:(t + 1) * 256]
                                    .rearrange("p (h w) -> p h w", w=16),
                                wv[:, 0:32:2, :], wv[:, 1:32:2, :])
                        # channel bias (commutes with spatial max)
                        nc.vector.tensor_scalar_add(dst[:], dst[:], bias_t[:, 0:1])

                    # gT: transpose pooled g, 128x128 chunks
                    for kt in range(KT):
                        pst = psA.tile([128, 128], BF16, tag="tr", bufs=2,
                                       name="pstr")
                        nc.tensor.transpose(
                            pst[:], g_p[:, kt * 128:(kt + 1) * 128], ident[:])
                        nc.vector.tensor_copy(
                            gT[:, kt * 128:(kt + 1) * 128], pst[:])

                # ---- stage B: attention + per-block tail ----
                with tc.tile_pool(name="psS", bufs=2, space="PSUM") as psS, \
                     tc.tile_pool(name="psY", bufs=1, space="PSUM") as psY, \
                     tc.tile_pool(name="psSum", bufs=1, space="PSUM") as psM, \
                     tc.tile_pool(name="psT", bufs=1, space="PSUM") as psT, \
                     tc.tile_pool(name="ptp", bufs=3) as ptp, \
                     tc.tile_pool(name="rcp", bufs=2) as rcp, \
                     tc.tile_pool(name="outp", bufs=3) as outp:
                    nc.vector.memset(sums_sb[:], 1.0)

                    def s_mms(qb, kt):
                        """scores S^T for (qb, kt) -> fresh psS tile"""
                        q0 = qb * QB
                        ss = psS.tile([128, QB], FP32, tag="S", name="ss")
                        for h in range(2):
                            nc.tensor.matmul(
                                ss[:, h * QC:(h + 1) * QC],
                                phi_p[:, kt * 128:kt * 128 + 128],
                                theta[:, q0 + h * QC:q0 + (h + 1) * QC],
                                start=True, stop=True)
                        return ss

                    ss = s_mms(0, 0)
                    for qb in range(NQB):
                        q0 = qb * QB
                        ya = psY.tile([128, QB], FP32, tag="y", name="ya")
                        s01 = psM.tile([64, QC], FP32, tag="sum", name="s01")
                        for kt in range(KT):
                            k0 = kt * 128
                            ptt = ptp.tile([128, QB], BF16, tag="pt", name="ptt")
                            nc.scalar.activation(ptt[:], ss[:], AF.Exp)
                            # software pipeline: next scores while ACT exps
                            if kt + 1 < KT:
                                ss = s_mms(qb, kt + 1)
                            elif qb + 1 < NQB:
                                ss = s_mms(qb + 1, 0)
                            for h in range(2):
                                nc.tensor.matmul(
                                    ya[:, h * QC:(h + 1) * QC],
                                    gT[:, k0:k0 + 128],
                                    ptt[:, h * QC:(h + 1) * QC],
                                    start=(kt == 0), stop=(kt == KT - 1))
                            nc.tensor.matmul(
                                s01[0:1, :], ones[:, 0:1], ptt[:, 0:QC],
                                start=(kt == 0), stop=(kt == KT - 1))
                            nc.tensor.matmul(
                                s01[32:33, :], ones[:, 0:1], ptt[:, QC:QB],
                                start=(kt == 0), stop=(kt == KT - 1),
                                tile_position=(0, 32))

                        # ---- per-block tail (overlaps next block's attention)
                        for j in range(2):
                            nc.vector.tensor_copy(
                                sums_sb[0:1, q0 + j * QC:q0 + (j + 1) * QC],
                                s01[32 * j:32 * j + 1, :])
                        nc.scalar.copy(yraw[:, q0:q0 + QB], ya[:])
                        # exact 1/sums: spread across 32 lanes via 32x32 block
                        # transpose so the 8-cycle/elem divide runs 32-wide
                        sp1 = rcp.tile([32, QB], FP32, tag="sp", name="sp1")
                        sp2 = rcp.tile([32, QB], FP32, tag="sph", name="sp2")
                        nc.vector.transpose(sp1[:], sums_sb[0:32, q0:q0 + QB])
                        nc.vector.memset(sp2[:], 1.0)
                        nc.vector.reciprocal(sp2[:, 0:QB:32], sp1[:, 0:QB:32])
                        nc.vector.transpose(rrow[0:32, q0:q0 + QB], sp2[:])
                        for j in range(2):
                            qs = slice(q0 + j * QC, q0 + (j + 1) * QC)
                            rps = psT.tile([128, QC], FP32, tag="t", name="rps")
                            nc.tensor.matmul(rps[:], onesf[0:1, :],
                                             rrow[0:1, qs], start=True, stop=True)
                            nc.vector.tensor_mul(yT[:, qs], yraw[:, qs], rps[:])
                            for oc in range(2):
                                pf = psT.tile([128, QC], FP32, tag="t", name="pf")
                                nc.tensor.matmul(
                                    pf[:], wfin[:, oc * 128:(oc + 1) * 128],
                                    yT[:, qs], start=True, stop=True)
                                ot = outp.tile([128, QC], FP32, tag="ot",
                                               name="ot")
                                nc.vector.tensor_add(ot[:], xres[oc][:, qs],
                                                     pf[:])
                                nc.sync.dma_start(out_d[oc, :, qs], ot[:])

            if loop_n:
                with tc.For_i(0, loop_n, 1):
                    body()
            else:
                body()

    split_excess_waits(nc)
    return nc


def prep_inputs(x, w_theta, b_theta, w_phi, b_phi, w_g, b_g,
                w_final, b_final, bn_gamma, bn_beta, bn_mean, bn_var):
    """Host-side prep -> list of 8 per-core input dicts."""
    BN_EPS = 1e-5
    x = np.asarray(x, np.float32)
    inv = (np.asarray(bn_gamma, np.float32)
           / np.sqrt(np.asarray(bn_var, np.float32) + BN_EPS))
    # BN(conv + b_final) = inv*conv + (inv*(b_final - mean) + beta)
    bias_tot = inv * (np.asarray(b_final, np.float32)
                      - np.asarray(bn_mean, np.float32)) + np.asarray(bn_beta, np.float32)
    wfinT = (np.asarray(w_final, np.float32) * inv[:, None]).T  # [Ci, C]

    def cchunk(a2d):  # [C, K] -> [2, 128, K]
        return np.ascontiguousarray(a2d.reshape(2, 128, -1))

    wthT = np.ascontiguousarray(np.asarray(w_theta, np.float32).T)  # [C, Ci]
    wphT = np.ascontiguousarray(np.asarray(w_phi, np.float32).T)
    wgT = np.ascontiguousarray(np.asarray(w_g, np.float32).T)

    common = {
        "wthT": cchunk(wthT).astype(FP16NP),
        "wphT": cchunk(wphT).astype(FP16NP),
        "wgT": cchunk(wgT).astype(FP16NP),
        "wfinT": np.ascontiguousarray(wfinT).astype(FP16NP),
        "b_th": np.asarray(b_theta, np.float32).reshape(128, 1),
        "b_ph": np.asarray(b_phi, np.float32).reshape(128, 1),
        "b_g": np.asarray(b_g, np.float32).reshape(128, 1),
        "ones_bf": np.ones((128, 128), BF16NP),
        "ones_f": np.ones((1, 128), np.float32),
        "ident": np.eye(128, dtype=BF16NP),
    }
    in_maps = []
    for c in range(8):
        b, s = c // 2, c % 2
        xf = x[b].reshape(C, N)
        # rotate so this core's slab occupies columns 0:NL (t-slice rotation
        # only permutes pooled keys; attention is key-permutation invariant)
        xrot = np.concatenate([xf[:, s * NL:(s + 1) * NL],
                               xf[:, (1 - s) * NL:(2 - s) * NL]], axis=1)
        m = dict(common)
        m["xb"] = cchunk(xrot).astype(FP16NP)
        m["xres"] = np.ascontiguousarray(
            (xrot[:, :NL] + bias_tot[:, None]).reshape(2, 128, NL)
        ).astype(np.float32)
        in_maps.append(m)
    return in_maps


def assemble(results):
    """results: list of 8 dicts with 'out' [2,128,NL] f32 -> [B,C,T,H,W]."""
    out = np.empty((B, C, N), np.float32)
    for c in range(8):
        b, s = c // 2, c % 2
        out[b, :, s * NL:(s + 1) * NL] = results[c]["out"].reshape(C, NL)
    return out.reshape(B, C, T, H, W)


_NC_CACHE = {}


def kernel(**inputs):
    from concourse.bass_utils import run_bass_kernel_spmd
    if "main" not in _NC_CACHE:
        _NC_CACHE["main"] = build_nc()
    nc = _NC_CACHE["main"]
    in_maps = prep_inputs(**inputs)
    res = run_bass_kernel_spmd(nc, in_maps, list(range(8)))
    return assemble(res.results)
